# revision 2
# baseline (speedup 1.0000x reference)
"""Trainium2 Bass kernel for nn_Pooling_block (B=128, N=785, C=384, pp=2).

Pure data-parallel over batch: 16 batches per core x 8 NeuronCores.

Per-core pipeline (per batch):
  - x nodes loaded as one 4-way patch-gathered DMA G[196, 4, 384]; the 4
    free-slots hold node-row offsets {0, 1, 28, 29} of each 2x2 patch
    (partition index = patch id ij, row-major). Split 98/98 on i-boundaries.
  - edge cast-loaded to bf16 fold tiles; partition-summed via ones-matmul on
    PE -> edge mean (the whole mean/ci/scores chain feeds sigmoids, so
    reduced precision is provably safe).
  - A_q = G[:,q] + G[:,q+2] vertical pair sums (DVE, f32r out).
  - node mean = ones-matmul over A tiles (f32r) = sum over all nodes.
  - ci = (sig(edge_mean) + sig(node_mean)) @ W_lin.T via f32r matmuls.
  - scores_k = rowwise dot(G_k, ci): fused DVE tensor_tensor_reduce in bf16
    against a PE-broadcast ci row; sigmoid on ACT -> per-patch T columns.
  - pooled = A_0 *col T0 + A_1 *col T1 (tensor_scalar / scalar_tensor_tensor).
  - A.T via PE transpose-mode (f32r) -> c-major A_cm [384, 197].
  - out = A_cm.T @ W_out_cls.T via f32r matmuls (f32r ~ 1.6e-4 rel err).
"""
import os
import sys

sys.path.insert(0, "/opt/trn_rl_repo")

import numpy as np

import concourse.bass as bass
import concourse.tile as tile
from concourse import bacc, mybir
from concourse.bass_utils import run_bass_kernel_spmd

B, N, C = 128, 785, 384
HW = N - 1          # 784
H = 28              # grid side
HP = 14             # pooled grid side
NPATCH = HP * HP    # 196
NB = 16             # batches per core
NCORES = 8
NOUT = 1 + NPATCH   # 197
CO = 2 * C          # 768

F32 = mybir.dt.float32
F32R = mybir.dt.float32r
BF16 = mybir.dt.bfloat16
FP16 = mybir.dt.float16
ADD = mybir.AluOpType.add
MUL = mybir.AluOpType.mult


KSTAGE = int(os.environ.get("KSTAGE", "99"))


def build_program(w_scalars):
    """Build the per-core SPMD program. w_scalars = (w00, w01, w10, w11) when
    the per-patch weights are channel-uniform, else None (general path)."""
    nc = bacc.Bacc(None, target_bir_lowering=False, debug=False)

    x_d = nc.declare_dram_parameter("x", [NB, N, C], F32, isOutput=False)
    e_d = nc.declare_dram_parameter("edge", [NB, N, C], F32, isOutput=False)
    wlt_d = nc.declare_dram_parameter("wlt", [C, C], F32, isOutput=False)
    wct_d = nc.declare_dram_parameter("wct", [C, CO], F32, isOutput=False)
    id_d = nc.declare_dram_parameter("ident", [128, 128], F32, isOutput=False)
    clsc_d = nc.declare_dram_parameter("cls_cm", [128, 3, NB], F32, isOutput=False)
    if w_scalars is None:
        wqr_d = nc.declare_dram_parameter("wqr", [4, 128, C], F32, isOutput=False)
    out_d = nc.declare_dram_parameter("out", [NB, NOUT, CO], F32, isOutput=True)

    # gathered view of x nodes: row n = 56i + 2j + (28p + q);  slot k = 2p+q
    x_nodes = [
        x_d[b, 1:N, :].rearrange("(i p j q) c -> i j p q c", i=HP, p=2, j=HP, q=2)
        for b in range(NB)
    ]

    with tile.TileContext(nc) as tc:
        with (
            tc.tile_pool(name="const", bufs=1) as cpool,
            tc.tile_pool(name="gx", bufs=3) as gxp,
            tc.tile_pool(name="gbf", bufs=2) as gbfp,
            tc.tile_pool(name="ed", bufs=3) as edp,
            tc.tile_pool(name="work", bufs=2) as wk,
            tc.tile_pool(name="small", bufs=2) as sm,
            tc.tile_pool(name="acm", bufs=2) as acmp,
            tc.tile_pool(name="ost", bufs=2) as ostp,
            tc.tile_pool(name="psA", bufs=1, space="PSUM") as psA,
            tc.tile_pool(name="psB", bufs=2, space="PSUM") as psB,
        ):
            # ---- constants ----
            ones_f = cpool.tile([128, 1], F32)
            nc.vector.memset(ones_f[:], 1.0)
            ones_r = cpool.tile([128, 1], F32R)
            nc.vector.tensor_copy(ones_r[:], ones_f[:])
            ones_bf_col = cpool.tile([128, 1], BF16)
            nc.vector.memset(ones_bf_col[:], 1.0)
            ones_bf_row = cpool.tile([1, 128], BF16)
            nc.vector.memset(ones_bf_row[:], 1.0)
            ones_f_row = cpool.tile([1, 128], F32)
            nc.vector.memset(ones_f_row[:], 1.0)
            one_f_11 = cpool.tile([1, 1], F32)
            nc.vector.memset(one_f_11[:], 1.0)

            ident_f = cpool.tile([128, 128], F32)
            nc.sync.dma_start(ident_f[:], id_d[:])
            ident_r = cpool.tile([128, 128], F32R)
            nc.vector.tensor_copy(ident_r[:], ident_f[:])

            wlt_r = []
            for cch in range(3):
                t = cpool.tile([128, C], F32, tag=f"wlt{cch}")
                nc.sync.dma_start(t[:], wlt_d[128 * cch : 128 * (cch + 1), :])
                tr = cpool.tile([128, C], F32R, tag=f"wltr{cch}")
                nc.vector.tensor_copy(tr[:], t[:])
                wlt_r.append(tr)
            wct_r = []
            for cch in range(3):
                t = cpool.tile([128, CO], F32, tag=f"wct{cch}")
                nc.sync.dma_start(t[:], wct_d[128 * cch : 128 * (cch + 1), :])
                tr = cpool.tile([128, CO], F32R, tag=f"wctr{cch}")
                nc.vector.tensor_copy(tr[:], t[:])
                wct_r.append(tr)
            if w_scalars is None:
                wqr_t = []
                for k in range(4):
                    t = cpool.tile([128, C], F32, tag=f"wqr{k}")
                    nc.sync.dma_start(t[:], wqr_d[k])
                    wqr_t.append(t)

            wqr_row = None
            if w_scalars is not None and len(set(w_scalars)) > 1:
                wqr_row = cpool.tile([128, 4], F32)
                for k in range(4):
                    nc.vector.memset(wqr_row[:, k : k + 1], float(w_scalars[k]))

            cls_cm = cpool.tile([128, 3, NB], F32)
            nc.sync.dma_start(cls_cm[:], clsc_d[:])

            # ---- per-batch pipeline ----
            for b in range(NB):
                # -- loads --
                g = []
                for t_i, (i0, i1) in enumerate(((0, 7), (7, 14))):
                    gt = gxp.tile([98, 4, C], F32, tag=f"g{t_i}")
                    for pp in range(2):
                        nc.scalar.dma_start(
                            gt[:, 2 * pp : 2 * pp + 2, :],
                            x_nodes[b][i0:i1, :, pp],
                        )
                    g.append(gt)

                efold = edp.tile([128, 6 * C], BF16, tag="efold")
                nc.gpsimd.dma_start(
                    efold[:], e_d[b, 0:768, :].rearrange("(p k) c -> p (k c)", p=128)
                )
                etail = edp.tile([17, C], BF16, tag="etail")
                nc.gpsimd.dma_start(etail[:], e_d[b, 768:785, :])

                # -- edge sums (bf16 ones-matmul, PSUM accumulate) --
                if KSTAGE < 2:
                    continue
                es = psA.tile([1, C], F32, tag="es")
                for k in range(6):
                    nc.tensor.matmul(
                        es[:], ones_bf_col[:], efold[:, C * k : C * (k + 1)],
                        start=(k == 0), stop=False,
                    )
                nc.tensor.matmul(
                    es[:], ones_bf_col[0:17, :], etail[:], start=False, stop=True
                )

                # -- fp16 casts of G for the scores chain (DVE 2x mode) --
                if KSTAGE < 3:
                    continue
                gbf = []
                for t_i in range(2):
                    gb = gbfp.tile([98, 4, C], FP16, tag=f"gbf{t_i}")
                    nc.vector.tensor_copy(gb[:], g[t_i][:])
                    gbf.append(gb)

                # -- vertical pair sums A_q (f32r) --
                a_t = {}
                for q in range(2):
                    for t_i in range(2):
                        at = wk.tile([98, C], F32R, tag=f"a{q}{t_i}")
                        nc.vector.tensor_add(at[:], g[t_i][:, q, :], g[t_i][:, 2 + q, :])
                        a_t[(q, t_i)] = at

                # -- node sums: ones-matmul over the 4 A tiles (f32r) --
                ns = psA.tile([1, C], F32, tag="ns")
                first = True
                for q in range(2):
                    for t_i in range(2):
                        nc.tensor.matmul(
                            ns[:], ones_r[0:98, :], a_t[(q, t_i)][:],
                            start=first, stop=(q == 1 and t_i == 1),
                        )
                        first = False

                # -- means -> sigmoid -> s row (f32r) --
                if KSTAGE < 4:
                    continue
                se = sm.tile([1, C], F32, tag="se")
                nc.scalar.activation(
                    se[:], es[:], mybir.ActivationFunctionType.Sigmoid, scale=1.0 / N
                )
                sn = sm.tile([1, C], F32, tag="sn")
                nc.scalar.activation(
                    sn[:], ns[:], mybir.ActivationFunctionType.Sigmoid, scale=1.0 / HW
                )
                s_row = sm.tile([1, C], F32, tag="srow")
                nc.vector.tensor_add(s_row[:], se[:], sn[:])

                # -- s row -> s col; ci row = s @ W_lin.T --
                s_colp = psA.tile([128, 3], F32, tag="smallp")
                for cch in range(3):
                    nc.tensor.matmul(
                        s_colp[:, cch : cch + 1],
                        s_row[:, 128 * cch : 128 * (cch + 1)],
                        one_f_11[:], start=True, stop=True,
                    )
                s_col = sm.tile([128, 3], F32R, tag="scol")
                nc.vector.tensor_copy(s_col[:], s_colp[:])

                cirow_p = psA.tile([1, C], F32, tag="smallp")
                for cch in range(3):
                    nc.tensor.matmul(
                        cirow_p[:], s_col[:, cch : cch + 1], wlt_r[cch][:],
                        start=(cch == 0), stop=(cch == 2),
                    )
                ci_f = sm.tile([1, C], F32, tag="cif")
                nc.scalar.copy(ci_f[:], cirow_p[:])

                # -- broadcast ci to 128 partitions (K=1 fp32 matmul; PE turns
                # fp16 operands into bf16 internally, so broadcast in fp32 and
                # round to fp16 only on the final SBUF copy) --
                cib_p = psA.tile([128, C], F32, tag="cibp")
                nc.tensor.matmul(cib_p[:], ones_f_row[:], ci_f[:], start=True, stop=True)
                cib_bf = sm.tile([128, C], FP16, tag="cibbf")
                nc.scalar.copy(cib_bf[:], cib_p[:])

                # -- scores: fused mul+reduce per gather slot (bf16, DVE) --
                if KSTAGE < 5:
                    continue
                s_acc = []
                for t_i in range(2):
                    sa = sm.tile([98, 4], F32, tag=f"sacc{t_i}")
                    s_acc.append(sa)
                    for k in range(4):
                        scr = wk.tile([98, C], FP16, tag="ttrs")
                        nc.vector.scalar_tensor_tensor(
                            scr[:], gbf[t_i][:, k, :], 1.0, cib_bf[0:98, :],
                            MUL, MUL, accum_out=sa[:, k : k + 1],
                        )

                sig = []
                for t_i in range(2):
                    sg = sm.tile([98, 4], F32, tag=f"sig{t_i}")
                    nc.scalar.activation(
                        sg[:], s_acc[t_i][:], mybir.ActivationFunctionType.Sigmoid
                    )
                    sig.append(sg)

                # -- pooled tiles (n-major [98, C] f32r) --
                if KSTAGE < 6:
                    continue
                pooled = []
                if w_scalars is not None:
                    w00, w01, w10, w11 = w_scalars
                    uniform_w = w00 == w01 == w10 == w11
                    for t_i in range(2):
                        # sp = sigma + 1; wsig = sp * w (skipped if all w equal);
                        # T01[:, q] = wsig[:, 2q] + wsig[:, 2q+1]
                        sp = sm.tile([98, 4], F32, tag=f"sp{t_i}")
                        nc.vector.tensor_scalar_add(sp[:], sig[t_i][:], 1.0)
                        if not uniform_w:
                            nc.vector.tensor_mul(sp[:], sp[:], wqr_row[0:98, :])
                        t01 = sm.tile([98, 2], F32, tag=f"t01_{t_i}")
                        nc.vector.tensor_add(
                            t01[:], sp[:, 0:4:2], sp[:, 1:4:2]
                        )
                        if uniform_w and w00 != 1.0:
                            nc.vector.tensor_scalar_mul(t01[:], t01[:], float(w00))
                        p0 = wk.tile([98, C], F32R, tag=f"p0_{t_i}")
                        nc.vector.tensor_scalar_mul(
                            p0[:], a_t[(0, t_i)][:], t01[:, 0:1]
                        )
                        pl = wk.tile([98, C], F32R, tag=f"pool{t_i}")
                        nc.vector.scalar_tensor_tensor(
                            pl[:], a_t[(1, t_i)][:], t01[:, 1:2], p0[:], MUL, ADD
                        )
                        pooled.append(pl)
                else:
                    for t_i in range(2):
                        sp = sm.tile([98, 4], F32, tag=f"sp{t_i}")
                        nc.vector.tensor_scalar_add(sp[:], sig[t_i][:], 1.0)
                        acc = None
                        for q in range(2):
                            for r in range(2):
                                k = 2 * q + r
                                bqr = wk.tile([98, C], F32, tag=f"bqr{t_i}")
                                nc.vector.tensor_scalar_mul(
                                    bqr[:], a_t[(q, t_i)][:], sp[:, k : k + 1]
                                )
                                term = wk.tile([98, C], F32, tag=f"term{t_i}")
                                nc.vector.tensor_mul(term[:], bqr[:], wqr_t[k][0:98, :])
                                if acc is None:
                                    acc = term
                                    # rotate tags so term and acc don't collide
                                else:
                                    nacc = wk.tile(
                                        [98, C], F32R if k == 3 else F32,
                                        tag=f"pacc{t_i}_{k % 2}",
                                    )
                                    nc.vector.tensor_add(nacc[:], acc[:], term[:])
                                    acc = nacc
                        pooled.append(acc)

                # -- c-major A: cls col + transposed pooled --
                if KSTAGE < 7:
                    continue
                a_cm = []
                for cch in range(3):
                    acm = acmp.tile([128, NOUT], F32R, tag=f"acm{cch}")
                    a_cm.append(acm)
                    nc.scalar.copy(acm[:, 0:1], cls_cm[:, cch, b : b + 1])
                    for t_i in range(2):
                        tp = psB.tile([128, 98], F32R, tag="tp")
                        nc.tensor.transpose(
                            tp[:],
                            pooled[t_i][:, 128 * cch : 128 * (cch + 1)],
                            ident_r[0:98, 0:98],
                        )
                        nc.scalar.copy(acm[:, 1 + 98 * t_i : 1 + 98 * (t_i + 1)], tp[:])

                # -- final matmul: out[row, co] = A_cm.T @ W_out_cls.T --
                if KSTAGE < 8:
                    continue
                for rch, (r0, rn) in enumerate(((0, 128), (128, 69))):
                    stile = ostp.tile([128, CO], F32, tag=f"ost{rch}")
                    for nh in range(2):
                        fo = psB.tile([128, C], F32, tag="fo")
                        for cch in range(3):
                            nc.tensor.matmul(
                                fo[0:rn, :],
                                a_cm[cch][:, r0 : r0 + rn],
                                wct_r[cch][:, C * nh : C * (nh + 1)],
                                start=(cch == 0), stop=(cch == 2),
                            )
                        nc.scalar.copy(stile[0:rn, C * nh : C * (nh + 1)], fo[0:rn, :])
                    nc.sync.dma_start(out_d[b, r0 : r0 + rn, :], stile[0:rn, :])

    nc.compile()
    return nc


def prepare(x, edge, W_lin, W_out_cls, weights):
    """Host-side prep shared by kernel() and the timing harness: returns
    (w_scalars, in_maps)."""
    x = np.ascontiguousarray(x, dtype=np.float32)
    edge = np.ascontiguousarray(edge, dtype=np.float32)
    wlt = np.ascontiguousarray(np.asarray(W_lin).T, dtype=np.float32)
    wct = np.ascontiguousarray(np.asarray(W_out_cls).T, dtype=np.float32)
    w = np.asarray(weights, dtype=np.float32)

    c_uniform = bool(np.all(w == w[0:1]))
    w_scalars = tuple(float(v) for v in w[0].reshape(4)) if c_uniform else None

    ident = np.eye(128, dtype=np.float32)
    in_maps = []
    for core in range(NCORES):
        sl = slice(core * NB, (core + 1) * NB)
        cls_cm = np.ascontiguousarray(
            x[sl, 0, :].T.reshape(3, 128, NB).transpose(1, 0, 2), dtype=np.float32
        )
        m = {
            "x": x[sl], "edge": edge[sl], "wlt": wlt, "wct": wct, "ident": ident,
            "cls_cm": cls_cm,
        }
        if w_scalars is None:
            wqr = np.empty((4, 128, C), dtype=np.float32)
            for q in range(2):
                for r in range(2):
                    wqr[2 * q + r] = np.broadcast_to(w[:, q, r], (128, C))
            m["wqr"] = wqr
        in_maps.append(m)
    return w_scalars, in_maps


def kernel(x, edge, W_lin, W_out_cls, weights):
    w_scalars, in_maps = prepare(x, edge, W_lin, W_out_cls, weights)
    nc = build_program(w_scalars)
    res = run_bass_kernel_spmd(nc, in_maps, list(range(NCORES)))
    out = np.concatenate([r["out"] for r in res.results], axis=0)
    return out



# revision 14
# speedup vs baseline: 1.0084x; 1.0084x over previous
"""Trainium2 Bass kernel for nn_Pooling_block (B=128, N=785, C=384, pp=2).

Pure data-parallel over batch: 16 batches per core x 8 NeuronCores.

v2 design (memory-regime; DMA floor ~135us/core):
  - All DRAM inputs declared float32r (same bits as f32) so every load runs
    on HWDGE with no cast DMAs and feeds PE matmuls at full f32r rate.
  - G[b] = patch-gathered x nodes [98, 4, C], ONE dma_start per half
    (196 descriptors x 3072 B contiguous lines).
  - edge folded [128, 6C] + [17, C]; per-batch sums via ones-matmuls that
    write DIRECTLY into row (b%4) of a [4, C] PSUM tile, so the whole
    sigmoid->W_lin->ci chain runs once per GROUP of 4 batches:
    sigmoid [4,C] -> add -> 3 PE transposes -> 3 matmuls -> ci_all [4, C].
  - ci row broadcast to 128 partitions via gpsimd partition_broadcast
    (SBUF->SBUF, keeps PE/PSUM out of it).
  - scores: fused DVE scalar_tensor_tensor in f32 (no fp16 casts at all).
  - pooled emitted in bf16 -> bf16 PE transposes -> bf16 final matmuls
    against W_out_cls.T (K=384 fp32 accumulation; ~2e-3 rel err, gate 2e-2).
  - PSUM: 8 banks exactly (es4, ns4, scolT, cip, tp2 x2, fo x2).
"""
import os
import sys

sys.path.insert(0, "/opt/trn_rl_repo")

import numpy as np

import concourse.bass as bass
import concourse.tile as tile
from concourse import bacc, mybir
from concourse.bass_utils import run_bass_kernel_spmd

B, N, C = 128, 785, 384
HW = N - 1          # 784
H = 28              # grid side
HP = 14             # pooled grid side
NPATCH = HP * HP    # 196
NB = 16             # batches per core
NCORES = 8
NOUT = 1 + NPATCH   # 197
CO = 2 * C          # 768
GRP = 4             # batches per chain group
NGRP = NB // GRP

F32 = mybir.dt.float32
F32R = mybir.dt.float32r
BF16 = mybir.dt.bfloat16
ADD = mybir.AluOpType.add
MUL = mybir.AluOpType.mult
SIGMOID = mybir.ActivationFunctionType.Sigmoid


def build_program(w_scalars):
    """Build the per-core SPMD program. w_scalars = (w00, w01, w10, w11) when
    the per-patch weights are channel-uniform, else None (general path)."""
    nc = bacc.Bacc(None, target_bir_lowering=False, debug=False)

    x_d = nc.declare_dram_parameter("x", [NB, N, C], F32R, isOutput=False)
    e_d = nc.declare_dram_parameter("edge", [NB, N, C], F32R, isOutput=False)
    wlt_d = nc.declare_dram_parameter("wlt", [C, C], F32R, isOutput=False)
    wct_d = nc.declare_dram_parameter("wct", [C, CO], F32, isOutput=False)
    id_d = nc.declare_dram_parameter("ident", [128, 128], F32R, isOutput=False)
    clsc_d = nc.declare_dram_parameter("cls_cm", [128, 3, NB], F32, isOutput=False)
    if w_scalars is None:
        wqr_d = nc.declare_dram_parameter("wqr", [4, 128, C], F32, isOutput=False)
    out_d = nc.declare_dram_parameter("out", [NB, NOUT, CO], F32, isOutput=True)

    # gathered view of x nodes: row n = 56i + 2j + (28p + q);  slot k = 2p+q
    x_nodes = [
        x_d[b, 1:N, :].rearrange("(i p j q) c -> i j p q c", i=HP, p=2, j=HP, q=2)
        for b in range(NB)
    ]

    uniform_w = w_scalars is not None and len(set(w_scalars)) == 1

    with tile.TileContext(nc) as tc:
        with (
            tc.tile_pool(name="const", bufs=1) as cpool,
            tc.tile_pool(name="gx", bufs=6) as gxp,
            tc.tile_pool(name="ed", bufs=3) as edp,
            tc.tile_pool(name="apool", bufs=6) as ap,
            tc.tile_pool(name="work", bufs=2) as wk,
            tc.tile_pool(name="small", bufs=2) as sm,
            tc.tile_pool(name="cibp", bufs=2) as cibp,
            tc.tile_pool(name="acm", bufs=2) as acmp,
            tc.tile_pool(name="ost", bufs=3) as ostp,
            tc.tile_pool(name="psE", bufs=1, space="PSUM") as psE,
            tc.tile_pool(name="psC", bufs=1, space="PSUM") as psC,
            tc.tile_pool(name="psT", bufs=2, space="PSUM") as psT,
            tc.tile_pool(name="psF", bufs=2, space="PSUM") as psF,
        ):
            # ---- constants ----
            ones_f = cpool.tile([128, 1], F32)
            nc.vector.memset(ones_f[:], 1.0)
            ones_r = cpool.tile([128, 1], F32R)
            nc.vector.tensor_copy(ones_r[:], ones_f[:])

            ident_r = cpool.tile([128, 128], F32R)
            nc.sync.dma_start(ident_r[:], id_d[:])
            ident_bf = cpool.tile([128, 128], BF16)
            nc.vector.tensor_copy(ident_bf[:], ident_r[:])

            wlt_r = []
            for cch in range(3):
                t = cpool.tile([128, C], F32R, tag=f"wlt{cch}")
                nc.sync.dma_start(t[:], wlt_d[128 * cch : 128 * (cch + 1), :])
                wlt_r.append(t)

            wct_bf = []
            for cch in range(3):
                stg = ostp.tile([128, CO], F32, tag="ost0")
                nc.sync.dma_start(stg[:], wct_d[128 * cch : 128 * (cch + 1), :])
                t = cpool.tile([128, CO], BF16, tag=f"wct{cch}")
                nc.vector.tensor_copy(t[:], stg[:])
                wct_bf.append(t)

            if w_scalars is None:
                wqr_t = []
                for k in range(4):
                    t = cpool.tile([128, C], F32, tag=f"wqr{k}")
                    nc.sync.dma_start(t[:], wqr_d[k])
                    wqr_t.append(t)

            cls_cm = cpool.tile([128, 3, NB], F32)
            nc.sync.dma_start(cls_cm[:], clsc_d[:])
            cls_bf = cpool.tile([128, 3, NB], BF16)
            nc.vector.tensor_copy(cls_bf[:], cls_cm[:])

            for g in range(NGRP):
                bs = range(g * GRP, (g + 1) * GRP)
                g_t, a_t = {}, {}

                # -- sub-loop 1: loads + per-batch token sums --
                scolT = psC.tile([128, 3, GRP], F32, tag="scolT")
                for b in bs:
                    gb = b % GRP
                    for t_i, (i0, i1) in enumerate(((0, 7), (7, 14))):
                        gt = gxp.tile([98, 4, C], F32R, tag=f"g{t_i}")
                        for pp in range(2):
                            nc.scalar.dma_start(
                                gt[:, 2 * pp : 2 * pp + 2, :],
                                x_nodes[b][i0:i1, :, pp],
                            )
                        g_t[(b, t_i)] = gt

                    efold = edp.tile([128, 6 * C], F32R, tag="efold")
                    nc.sync.dma_start(
                        efold[:],
                        e_d[b, 0:768, :].rearrange("(p k) c -> p (k c)", p=128),
                    )
                    etail = edp.tile([17, C], F32R, tag="etail")
                    nc.sync.dma_start(etail[:], e_d[b, 768:785, :])

                    # edge sums
                    es = psE.tile([1, C], F32, tag="es")
                    for k in range(6):
                        nc.tensor.matmul(
                            es[:],
                            ones_r[:],
                            efold[:, C * k : C * (k + 1)],
                            start=(k == 0), stop=False,
                        )
                    nc.tensor.matmul(
                        es[:], ones_r[0:17, :], etail[:], start=False, stop=True
                    )

                    # vertical pair sums A_q (f32r)
                    for t_i in range(2):
                        at = ap.tile([98, 2, C], F32R, tag=f"a{t_i}")
                        for q in range(2):
                            nc.vector.tensor_add(
                                at[:, q, :],
                                g_t[(b, t_i)][:, q, :],
                                g_t[(b, t_i)][:, 2 + q, :],
                            )
                        a_t[(b, t_i)] = at

                    # node sums
                    ns = psE.tile([1, C], F32, tag="ns")
                    first = True
                    for q in range(2):
                        for t_i in range(2):
                            nc.tensor.matmul(
                                ns[:],
                                ones_r[0:98, :],
                                a_t[(b, t_i)][:, q, :],
                                start=first, stop=(q == 1 and t_i == 1),
                            )
                            first = False

                    # sigmoid means -> s_row; transpose into column gb of scolT
                    se = sm.tile([1, C], F32, tag="se")
                    nc.scalar.activation(se[:], es[:], SIGMOID, scale=1.0 / N)
                    sn = sm.tile([1, C], F32, tag="sn")
                    nc.scalar.activation(sn[:], ns[:], SIGMOID, scale=1.0 / HW)
                    s_row = sm.tile([1, C], F32, tag="srow")
                    nc.vector.tensor_add(s_row[:], se[:], sn[:])
                    for cch in range(3):
                        nc.tensor.matmul(
                            scolT[:, cch, gb : gb + 1],
                            s_row[:, 128 * cch : 128 * (cch + 1)],
                            ones_f[0:1, :],
                            start=True, stop=True,
                        )

                # -- group chain: s columns for the whole group --
                scolT_sb = sm.tile([128, 3, GRP], F32R, tag="scolsb")
                nc.scalar.copy(scolT_sb[:], scolT[:])

                # -- sub-loop 2: scores / pooled / transpose / final / store --
                for b in bs:
                    gb = b % GRP
                    cip = psC.tile([1, C], F32, tag="cip")
                    for cch in range(3):
                        nc.tensor.matmul(
                            cip[:], scolT_sb[:, cch, gb : gb + 1], wlt_r[cch][:],
                            start=(cch == 0), stop=(cch == 2),
                        )
                    ci_b = sm.tile([1, C], F32R, tag="cirow")
                    nc.scalar.copy(ci_b[:], cip[:])
                    cib = cibp.tile([128, C], F32R, tag="cib")
                    nc.gpsimd.partition_broadcast(cib[:], ci_b[:])

                    sig = []
                    for t_i in range(2):
                        sa = sm.tile([98, 4], F32, tag=f"sacc{t_i}")
                        for k in range(4):
                            scr = wk.tile([98, C], F32, tag=f"scr{t_i}")
                            nc.vector.scalar_tensor_tensor(
                                scr[:], g_t[(b, t_i)][:, k, :], 1.0, cib[0:98, :],
                                MUL, MUL, accum_out=sa[:, k : k + 1],
                            )
                        sg = sm.tile([98, 4], F32, tag=f"sig{t_i}")
                        nc.scalar.activation(sg[:], sa[:], SIGMOID)
                        sig.append(sg)

                    pooled = []
                    for t_i in range(2):
                        sp = sm.tile([98, 4], F32, tag=f"sp{t_i}")
                        nc.vector.tensor_scalar_add(sp[:], sig[t_i][:], 1.0)
                        at = a_t[(b, t_i)]
                        if w_scalars is not None:
                            w00, w01, w10, w11 = w_scalars
                            if not uniform_w:
                                wrow = sm.tile([98, 4], F32, tag=f"wrow{t_i}")
                                for k in range(4):
                                    nc.vector.memset(
                                        wrow[:, k : k + 1], float(w_scalars[k])
                                    )
                                nc.vector.tensor_mul(sp[:], sp[:], wrow[:])
                            t01 = sm.tile([98, 2], F32, tag=f"t01_{t_i}")
                            nc.vector.tensor_add(t01[:], sp[:, 0:4:2], sp[:, 1:4:2])
                            if uniform_w and w00 != 1.0:
                                nc.vector.tensor_scalar_mul(t01[:], t01[:], float(w00))
                            p0 = wk.tile([98, C], F32, tag=f"p0_{t_i}")
                            nc.vector.tensor_scalar_mul(
                                p0[:], at[:, 0, :], t01[:, 0:1]
                            )
                            pl = wk.tile([98, C], BF16, tag=f"pl{t_i}")
                            nc.vector.scalar_tensor_tensor(
                                pl[:], at[:, 1, :], t01[:, 1:2], p0[:], MUL, ADD
                            )
                        else:
                            # general per-channel weights: m_q[98, C] then combine
                            mqs = []
                            for q in range(2):
                                m0 = wk.tile([98, C], F32, tag=f"mq{q}{t_i}a")
                                nc.vector.tensor_scalar_mul(
                                    m0[:], wqr_t[2 * q][0:98, :], sp[:, 2 * q : 2 * q + 1]
                                )
                                mq = wk.tile([98, C], F32, tag=f"mq{q}{t_i}b")
                                nc.vector.scalar_tensor_tensor(
                                    mq[:], wqr_t[2 * q + 1][0:98, :],
                                    sp[:, 2 * q + 1 : 2 * q + 2], m0[:], MUL, ADD,
                                )
                                mqs.append(mq)
                            p0 = wk.tile([98, C], F32, tag=f"p0_{t_i}")
                            nc.vector.tensor_mul(p0[:], at[:, 0, :], mqs[0][:])
                            p1 = wk.tile([98, C], F32, tag=f"p1_{t_i}")
                            nc.vector.tensor_mul(p1[:], at[:, 1, :], mqs[1][:])
                            pl = wk.tile([98, C], BF16, tag=f"pl{t_i}")
                            nc.vector.tensor_add(pl[:], p0[:], p1[:])
                        pooled.append(pl)

                    # c-major A via bf16 PE transposes + cls column
                    a_cm = []
                    for cch in range(3):
                        tp2 = psT.tile([128, 2 * 98], BF16, tag="tp2")
                        for t_i in range(2):
                            nc.tensor.transpose(
                                tp2[:, 98 * t_i : 98 * (t_i + 1)],
                                pooled[t_i][:, 128 * cch : 128 * (cch + 1)],
                                ident_bf[0:98, 0:98],
                            )
                        acm = acmp.tile([128, NOUT], BF16, tag=f"acm{cch}")
                        nc.scalar.copy(acm[:, 0:1], cls_bf[:, cch, b : b + 1])
                        nc.scalar.copy(acm[:, 1:NOUT], tp2[:])
                        a_cm.append(acm)

                    # final matmul: out[row, co] = A_cm.T @ W_out_cls.T (bf16)
                    for rch, (r0, rn) in enumerate(((0, 128), (128, 69))):
                        stile = ostp.tile([128, CO], F32, tag=f"ost{rch}")
                        for nh in range(2):
                            fo = psF.tile([128, C], F32, tag="fo")
                            for cch in range(3):
                                nc.tensor.matmul(
                                    fo[0:rn, :],
                                    a_cm[cch][:, r0 : r0 + rn],
                                    wct_bf[cch][:, C * nh : C * (nh + 1)],
                                    start=(cch == 0), stop=(cch == 2),
                                )
                            if nh == 0:
                                nc.scalar.copy(stile[0:rn, 0:C], fo[0:rn, :])
                            else:
                                nc.vector.tensor_copy(stile[0:rn, C:CO], fo[0:rn, :])
                        nc.sync.dma_start(out_d[b, r0 : r0 + rn, :], stile[0:rn, :])

    nc.compile()
    return nc


def prepare(x, edge, W_lin, W_out_cls, weights):
    """Host-side prep shared by kernel() and the timing harness: returns
    (w_scalars, in_maps)."""
    x = np.ascontiguousarray(x, dtype=np.float32)
    edge = np.ascontiguousarray(edge, dtype=np.float32)
    wlt = np.ascontiguousarray(np.asarray(W_lin).T, dtype=np.float32)
    wct = np.ascontiguousarray(np.asarray(W_out_cls).T, dtype=np.float32)
    w = np.asarray(weights, dtype=np.float32)

    c_uniform = bool(np.all(w == w[0:1]))
    w_scalars = tuple(float(v) for v in w[0].reshape(4)) if c_uniform else None

    ident = np.eye(128, dtype=np.float32)
    in_maps = []
    for core in range(NCORES):
        sl = slice(core * NB, (core + 1) * NB)
        cls_cm = np.ascontiguousarray(
            x[sl, 0, :].T.reshape(3, 128, NB).transpose(1, 0, 2), dtype=np.float32
        )
        m = {
            "x": x[sl], "edge": edge[sl], "wlt": wlt, "wct": wct, "ident": ident,
            "cls_cm": cls_cm,
        }
        if w_scalars is None:
            wqr = np.empty((4, 128, C), dtype=np.float32)
            for q in range(2):
                for r in range(2):
                    wqr[2 * q + r] = np.broadcast_to(w[:, q, r], (128, C))
            m["wqr"] = wqr
        in_maps.append(m)
    return w_scalars, in_maps


def kernel(x, edge, W_lin, W_out_cls, weights):
    w_scalars, in_maps = prepare(x, edge, W_lin, W_out_cls, weights)
    nc = build_program(w_scalars)
    res = run_bass_kernel_spmd(nc, in_maps, list(range(NCORES)))
    out = np.concatenate([r["out"] for r in res.results], axis=0)
    return out


# revision 18
# speedup vs baseline: 1.0263x; 1.0177x over previous
"""Trainium2 Bass kernel for nn_Pooling_block (B=128, N=785, C=384, pp=2).

Pure data-parallel over batch: 16 batches per core x 8 NeuronCores.

v2 design (memory-regime; DMA floor ~135us/core):
  - All DRAM inputs declared float32r (same bits as f32) so every load runs
    on HWDGE with no cast DMAs and feeds PE matmuls at full f32r rate.
  - G[b] = patch-gathered x nodes [98, 4, C], ONE dma_start per half
    (196 descriptors x 3072 B contiguous lines).
  - edge folded [128, 6C] + [17, C]; per-batch sums via ones-matmuls that
    write DIRECTLY into row (b%4) of a [4, C] PSUM tile, so the whole
    sigmoid->W_lin->ci chain runs once per GROUP of 4 batches:
    sigmoid [4,C] -> add -> 3 PE transposes -> 3 matmuls -> ci_all [4, C].
  - ci row broadcast to 128 partitions via gpsimd partition_broadcast
    (SBUF->SBUF, keeps PE/PSUM out of it).
  - scores: fused DVE scalar_tensor_tensor in f32 (no fp16 casts at all).
  - pooled emitted in bf16 -> bf16 PE transposes -> bf16 final matmuls
    against W_out_cls.T (K=384 fp32 accumulation; ~2e-3 rel err, gate 2e-2).
  - PSUM: 8 banks exactly (es4, ns4, scolT, cip, tp2 x2, fo x2).
"""
import os
import sys

sys.path.insert(0, "/opt/trn_rl_repo")

import numpy as np

import concourse.bass as bass
import concourse.tile as tile
from concourse import bacc, mybir
from concourse.bass_utils import run_bass_kernel_spmd

B, N, C = 128, 785, 384
HW = N - 1          # 784
H = 28              # grid side
HP = 14             # pooled grid side
NPATCH = HP * HP    # 196
NB = 16             # batches per core
NCORES = 8
NOUT = 1 + NPATCH   # 197
CO = 2 * C          # 768
GRP = 4             # batches per chain group
NGRP = NB // GRP

F32 = mybir.dt.float32
F32R = mybir.dt.float32r
BF16 = mybir.dt.bfloat16
ADD = mybir.AluOpType.add
MUL = mybir.AluOpType.mult
SIGMOID = mybir.ActivationFunctionType.Sigmoid


def build_program(w_scalars):
    """Build the per-core SPMD program. w_scalars = (w00, w01, w10, w11) when
    the per-patch weights are channel-uniform, else None (general path)."""
    nc = bacc.Bacc(None, target_bir_lowering=False, debug=False)

    x_d = nc.declare_dram_parameter("x", [NB, N, C], F32R, isOutput=False)
    e_d = nc.declare_dram_parameter("edge", [NB, N, C], F32R, isOutput=False)
    wlt_d = nc.declare_dram_parameter("wlt", [C, C], F32R, isOutput=False)
    wct_d = nc.declare_dram_parameter("wct", [C, CO], F32, isOutput=False)
    id_d = nc.declare_dram_parameter("ident", [128, 128], F32R, isOutput=False)
    clsc_d = nc.declare_dram_parameter("cls_cm", [128, 3, NB], F32, isOutput=False)
    if w_scalars is None:
        wqr_d = nc.declare_dram_parameter("wqr", [4, 128, C], F32, isOutput=False)
    out_d = nc.declare_dram_parameter("out", [NB, NOUT, CO], F32, isOutput=True)

    # gathered view of x nodes: row n = 56i + 2j + (28p + q);  slot k = 2p+q
    x_nodes = [
        x_d[b, 1:N, :].rearrange("(i p j q) c -> i j p q c", i=HP, p=2, j=HP, q=2)
        for b in range(NB)
    ]

    uniform_w = w_scalars is not None and len(set(w_scalars)) == 1

    with tile.TileContext(nc) as tc:
        with (
            tc.tile_pool(name="const", bufs=1) as cpool,
            tc.tile_pool(name="gx", bufs=6) as gxp,
            tc.tile_pool(name="ed", bufs=2) as edp,
            tc.tile_pool(name="apool", bufs=6) as ap,
            tc.tile_pool(name="work", bufs=2) as wk,
            tc.tile_pool(name="small", bufs=2) as sm,
            tc.tile_pool(name="cibp", bufs=1) as cibp,
            tc.tile_pool(name="acm", bufs=2) as acmp,
            tc.tile_pool(name="ost", bufs=2) as ostp,
            tc.tile_pool(name="psE", bufs=1, space="PSUM") as psE,
            tc.tile_pool(name="psC", bufs=1, space="PSUM") as psC,
            tc.tile_pool(name="psT", bufs=2, space="PSUM") as psT,
            tc.tile_pool(name="psF", bufs=2, space="PSUM") as psF,
        ):
            # ---- constants ----
            ones_f = cpool.tile([128, 1], F32)
            nc.vector.memset(ones_f[:], 1.0)
            ones_r = cpool.tile([128, 1], F32R)
            nc.vector.tensor_copy(ones_r[:], ones_f[:])

            ident_r = cpool.tile([128, 128], F32R)
            nc.sync.dma_start(ident_r[:], id_d[:])
            ident_bf = cpool.tile([128, 128], BF16)
            nc.vector.tensor_copy(ident_bf[:], ident_r[:])

            wlt_r = []
            for cch in range(3):
                t = cpool.tile([128, C], F32R, tag=f"wlt{cch}")
                nc.sync.dma_start(t[:], wlt_d[128 * cch : 128 * (cch + 1), :])
                wlt_r.append(t)

            wct_bf = []
            for cch in range(3):
                stg = ostp.tile([128, CO], F32, tag="ost0")
                nc.sync.dma_start(stg[:], wct_d[128 * cch : 128 * (cch + 1), :])
                t = cpool.tile([128, CO], BF16, tag=f"wct{cch}")
                nc.vector.tensor_copy(t[:], stg[:])
                wct_bf.append(t)

            if w_scalars is None:
                wqr_t = []
                for k in range(4):
                    t = cpool.tile([128, C], F32, tag=f"wqr{k}")
                    nc.sync.dma_start(t[:], wqr_d[k])
                    wqr_t.append(t)

            cls_cm = cpool.tile([128, 3, NB], F32)
            nc.sync.dma_start(cls_cm[:], clsc_d[:])
            cls_bf = cpool.tile([128, 3, NB], BF16)
            nc.vector.tensor_copy(cls_bf[:], cls_cm[:])

            for g in range(NGRP):
                bs = range(g * GRP, (g + 1) * GRP)
                g_t, a_t = {}, {}

                # -- sub-loop 1: loads + per-batch token sums --
                scolT = psC.tile([128, 3, GRP], F32, tag="scolT")
                for b in bs:
                    gb = b % GRP
                    # all loads issue from SP (sync): its stream has no compute,
                    # so slot-free waits never block compute instructions.
                    for t_i, (i0, i1) in enumerate(((0, 7), (7, 14))):
                        gt = gxp.tile([98, 4, C], F32R, tag=f"g{t_i}")
                        for pp in range(2):
                            nc.sync.dma_start(
                                gt[:, 2 * pp : 2 * pp + 2, :],
                                x_nodes[b][i0:i1, :, pp],
                            )
                        g_t[(b, t_i)] = gt

                    efold = edp.tile([128, 6 * C], F32R, tag="efold")
                    nc.sync.dma_start(
                        efold[:],
                        e_d[b, 0:768, :].rearrange("(p k) c -> p (k c)", p=128),
                    )
                    etail = edp.tile([17, C], F32R, tag="etail")
                    nc.sync.dma_start(etail[:], e_d[b, 768:785, :])

                    # edge sums
                    es = psE.tile([1, C], F32, tag="es")
                    for k in range(6):
                        nc.tensor.matmul(
                            es[:],
                            ones_r[:],
                            efold[:, C * k : C * (k + 1)],
                            start=(k == 0), stop=False,
                        )
                    nc.tensor.matmul(
                        es[:], ones_r[0:17, :], etail[:], start=False, stop=True
                    )

                    # vertical pair sums A_q (f32r)
                    for t_i in range(2):
                        at = ap.tile([98, 2, C], F32R, tag=f"a{t_i}")
                        for q in range(2):
                            nc.vector.tensor_add(
                                at[:, q, :],
                                g_t[(b, t_i)][:, q, :],
                                g_t[(b, t_i)][:, 2 + q, :],
                            )
                        a_t[(b, t_i)] = at

                    # node sums
                    ns = psE.tile([1, C], F32, tag="ns")
                    first = True
                    for q in range(2):
                        for t_i in range(2):
                            nc.tensor.matmul(
                                ns[:],
                                ones_r[0:98, :],
                                a_t[(b, t_i)][:, q, :],
                                start=first, stop=(q == 1 and t_i == 1),
                            )
                            first = False

                    # sigmoid means -> s_row; transpose into column gb of scolT
                    se = sm.tile([1, C], F32, tag="se")
                    nc.scalar.activation(se[:], es[:], SIGMOID, scale=1.0 / N)
                    sn = sm.tile([1, C], F32, tag="sn")
                    nc.scalar.activation(sn[:], ns[:], SIGMOID, scale=1.0 / HW)
                    s_row = sm.tile([1, C], F32, tag="srow")
                    nc.vector.tensor_add(s_row[:], se[:], sn[:])
                    for cch in range(3):
                        nc.tensor.matmul(
                            scolT[:, cch, gb : gb + 1],
                            s_row[:, 128 * cch : 128 * (cch + 1)],
                            ones_f[0:1, :],
                            start=True, stop=True,
                        )

                # -- group chain: s columns -> ci rows -> broadcasts up front --
                scolT_sb = sm.tile([128, 3, GRP], F32R, tag="scolsb")
                nc.scalar.copy(scolT_sb[:], scolT[:])

                cibs = []
                for gb in range(GRP):
                    cip = psC.tile([1, C], F32, tag="cip")
                    for cch in range(3):
                        nc.tensor.matmul(
                            cip[:], scolT_sb[:, cch, gb : gb + 1], wlt_r[cch][:],
                            start=(cch == 0), stop=(cch == 2),
                        )
                    ci_b = sm.tile([1, C], F32R, tag=f"cirow{gb}")
                    nc.scalar.copy(ci_b[:], cip[:])
                    cib = cibp.tile([128, C], F32R, tag=f"cib{gb}")
                    nc.gpsimd.partition_broadcast(cib[:], ci_b[:])
                    cibs.append(cib)

                # -- sub-loop 2: scores / pooled / transpose / final / store --
                for b in bs:
                    gb = b % GRP
                    cib = cibs[gb]

                    sig = []
                    for t_i in range(2):
                        sa = sm.tile([98, 4], F32, tag=f"sacc{t_i}")
                        for k in range(4):
                            scr = wk.tile([98, C], F32, tag=f"scr{t_i}")
                            nc.vector.scalar_tensor_tensor(
                                scr[:], g_t[(b, t_i)][:, k, :], 1.0, cib[0:98, :],
                                MUL, MUL, accum_out=sa[:, k : k + 1],
                            )
                        sg = sm.tile([98, 4], F32, tag=f"sig{t_i}")
                        nc.scalar.activation(sg[:], sa[:], SIGMOID)
                        sig.append(sg)

                    pooled = []
                    for t_i in range(2):
                        sp = sm.tile([98, 4], F32, tag=f"sp{t_i}")
                        nc.vector.tensor_scalar_add(sp[:], sig[t_i][:], 1.0)
                        at = a_t[(b, t_i)]
                        if w_scalars is not None:
                            w00, w01, w10, w11 = w_scalars
                            if not uniform_w:
                                wrow = sm.tile([98, 4], F32, tag=f"wrow{t_i}")
                                for k in range(4):
                                    nc.vector.memset(
                                        wrow[:, k : k + 1], float(w_scalars[k])
                                    )
                                nc.vector.tensor_mul(sp[:], sp[:], wrow[:])
                            t01 = sm.tile([98, 2], F32, tag=f"t01_{t_i}")
                            nc.vector.tensor_add(t01[:], sp[:, 0:4:2], sp[:, 1:4:2])
                            if uniform_w and w00 != 1.0:
                                nc.vector.tensor_scalar_mul(t01[:], t01[:], float(w00))
                            p0 = wk.tile([98, C], F32, tag=f"p0_{t_i}")
                            nc.vector.tensor_scalar_mul(
                                p0[:], at[:, 0, :], t01[:, 0:1]
                            )
                            pl = wk.tile([98, C], BF16, tag=f"pl{t_i}")
                            nc.vector.scalar_tensor_tensor(
                                pl[:], at[:, 1, :], t01[:, 1:2], p0[:], MUL, ADD
                            )
                        else:
                            # general per-channel weights: m_q[98, C] then combine
                            mqs = []
                            for q in range(2):
                                m0 = wk.tile([98, C], F32, tag=f"mq{q}{t_i}a")
                                nc.vector.tensor_scalar_mul(
                                    m0[:], wqr_t[2 * q][0:98, :], sp[:, 2 * q : 2 * q + 1]
                                )
                                mq = wk.tile([98, C], F32, tag=f"mq{q}{t_i}b")
                                nc.vector.scalar_tensor_tensor(
                                    mq[:], wqr_t[2 * q + 1][0:98, :],
                                    sp[:, 2 * q + 1 : 2 * q + 2], m0[:], MUL, ADD,
                                )
                                mqs.append(mq)
                            p0 = wk.tile([98, C], F32, tag=f"p0_{t_i}")
                            nc.vector.tensor_mul(p0[:], at[:, 0, :], mqs[0][:])
                            p1 = wk.tile([98, C], F32, tag=f"p1_{t_i}")
                            nc.vector.tensor_mul(p1[:], at[:, 1, :], mqs[1][:])
                            pl = wk.tile([98, C], BF16, tag=f"pl{t_i}")
                            nc.vector.tensor_add(pl[:], p0[:], p1[:])
                        pooled.append(pl)

                    # c-major A via bf16 PE transposes + cls column
                    a_cm = []
                    for cch in range(3):
                        tp2 = psT.tile([128, 2 * 98], BF16, tag="tp2")
                        for t_i in range(2):
                            nc.tensor.transpose(
                                tp2[:, 98 * t_i : 98 * (t_i + 1)],
                                pooled[t_i][:, 128 * cch : 128 * (cch + 1)],
                                ident_bf[0:98, 0:98],
                            )
                        acm = acmp.tile([128, NOUT], BF16, tag=f"acm{cch}")
                        nc.scalar.copy(acm[:, 0:1], cls_bf[:, cch, b : b + 1])
                        nc.scalar.copy(acm[:, 1:NOUT], tp2[:])
                        a_cm.append(acm)

                    # final matmul: out[row, co] = A_cm.T @ W_out_cls.T (bf16)
                    for rch, (r0, rn) in enumerate(((0, 128), (128, 69))):
                        stile = ostp.tile([128, CO], F32, tag=f"ost{rch}")
                        for nh in range(2):
                            fo = psF.tile([128, C], F32, tag="fo")
                            for cch in range(3):
                                nc.tensor.matmul(
                                    fo[0:rn, :],
                                    a_cm[cch][:, r0 : r0 + rn],
                                    wct_bf[cch][:, C * nh : C * (nh + 1)],
                                    start=(cch == 0), stop=(cch == 2),
                                )
                            # both halves on ACT so the store that follows in
                            # ACT's stream never waits on another engine
                            nc.scalar.copy(
                                stile[0:rn, C * nh : C * (nh + 1)], fo[0:rn, :]
                            )
                        nc.scalar.dma_start(out_d[b, r0 : r0 + rn, :], stile[0:rn, :])

    nc.compile()
    return nc


def prepare(x, edge, W_lin, W_out_cls, weights):
    """Host-side prep shared by kernel() and the timing harness: returns
    (w_scalars, in_maps)."""
    x = np.ascontiguousarray(x, dtype=np.float32)
    edge = np.ascontiguousarray(edge, dtype=np.float32)
    wlt = np.ascontiguousarray(np.asarray(W_lin).T, dtype=np.float32)
    wct = np.ascontiguousarray(np.asarray(W_out_cls).T, dtype=np.float32)
    w = np.asarray(weights, dtype=np.float32)

    c_uniform = bool(np.all(w == w[0:1]))
    w_scalars = tuple(float(v) for v in w[0].reshape(4)) if c_uniform else None

    ident = np.eye(128, dtype=np.float32)
    in_maps = []
    for core in range(NCORES):
        sl = slice(core * NB, (core + 1) * NB)
        cls_cm = np.ascontiguousarray(
            x[sl, 0, :].T.reshape(3, 128, NB).transpose(1, 0, 2), dtype=np.float32
        )
        m = {
            "x": x[sl], "edge": edge[sl], "wlt": wlt, "wct": wct, "ident": ident,
            "cls_cm": cls_cm,
        }
        if w_scalars is None:
            wqr = np.empty((4, 128, C), dtype=np.float32)
            for q in range(2):
                for r in range(2):
                    wqr[2 * q + r] = np.broadcast_to(w[:, q, r], (128, C))
            m["wqr"] = wqr
        in_maps.append(m)
    return w_scalars, in_maps


def kernel(x, edge, W_lin, W_out_cls, weights):
    w_scalars, in_maps = prepare(x, edge, W_lin, W_out_cls, weights)
    nc = build_program(w_scalars)
    res = run_bass_kernel_spmd(nc, in_maps, list(range(NCORES)))
    out = np.concatenate([r["out"] for r in res.results], axis=0)
    return out
